# revision 4
# baseline (speedup 1.0000x reference)
"""Trainium2 Bass kernel for DGMoLE (dense-gated mixture of LoRA experts).

Computes, for x:[B,S,Din], W_base:[Dout,Din], b_base:[Dout], W_router:[E,Din],
b_router:[E], lora_A:[E,Din,R], lora_B:[E,R,Dout]:

    base   = x @ W_base.T + b_base
    wts    = sparsemax(x @ W_router.T + b_router)
    h      = einsum('td,edr->ter', x, lora_A)
    out    = base + einsum('ter,te,ero->to', h, wts, lora_B)

Sharding over 8 NeuronCores: 4 token-quarters x 2 Dout-halves.  Each core
holds its W_base half-transposed in SBUF as bf16 and streams its token
quarter through it.  All matmuls run in bf16 with fp32 PSUM accumulation.
"""

import sys

sys.path.insert(0, "/opt/trn_rl_repo")

import numpy as np
import ml_dtypes

from concourse import bacc, tile, mybir
from concourse.bass_utils import run_bass_kernel_spmd

f32 = mybir.dt.float32
bf16 = mybir.dt.bfloat16
Add = mybir.AluOpType.add
Mult = mybir.AluOpType.mult
Max = mybir.AluOpType.max
Min = mybir.AluOpType.min
IsGt = mybir.AluOpType.is_gt
Sub = mybir.AluOpType.subtract

# Problem dims (hardcoded per spec).
B, S, D, O = 8, 2048, 4096, 4096
E, R = 8, 16
ER = E * R  # 128
N_CORES = 8
TQ = 4          # token quarters
OH = 2          # output halves
T_CORE = B * S // TQ      # 4096 tokens per core
O_CORE = O // OH          # 2048 output dims per core
NT = T_CORE // 128        # 32 token tiles
NC_D = D // 128           # 32 contraction chunks
NOG = 2                   # o-groups of 1024 per core
RH = 8 + ER               # router+h fused rhs width = 136

# Batcher odd-even mergesort network for 8 elements (descending).
SORT8 = [(0, 1), (2, 3), (4, 5), (6, 7),
         (0, 2), (1, 3), (4, 6), (5, 7),
         (1, 2), (5, 6),
         (0, 4), (1, 5), (2, 6), (3, 7),
         (2, 4), (3, 5),
         (1, 2), (3, 4), (5, 6)]

_CACHE = {}


def _build():
    if "nc" in _CACHE:
        return _CACHE["nc"]

    nc = bacc.Bacc("TRN2", target_bir_lowering=False, debug=False,
                   num_devices=N_CORES)
    x_d = nc.dram_tensor("x", [T_CORE, D], f32, kind="ExternalInput").ap()
    w_d = nc.dram_tensor("w", [O_CORE, D], f32, kind="ExternalInput").ap()
    b_d = nc.dram_tensor("b", [O_CORE], f32, kind="ExternalInput").ap()
    wr_d = nc.dram_tensor("wr", [E, D], f32, kind="ExternalInput").ap()
    br_d = nc.dram_tensor("br", [E], f32, kind="ExternalInput").ap()
    la_d = nc.dram_tensor("la", [E, D, R], f32, kind="ExternalInput").ap()
    lb_d = nc.dram_tensor("lb", [ER, O_CORE], f32, kind="ExternalInput").ap()
    id_d = nc.dram_tensor("ident", [128, 128], bf16, kind="ExternalInput").ap()
    out_d = nc.dram_tensor("out", [T_CORE, O_CORE], f32,
                           kind="ExternalOutput").ap()

    with tile.TileContext(nc) as tc:
        with (
            tc.tile_pool(name="const", bufs=1) as cpool,
            tc.tile_pool(name="stage", bufs=2) as stpool,
            tc.tile_pool(name="xt", bufs=2) as xtpool,
            tc.tile_pool(name="small", bufs=2) as smpool,
            tc.tile_pool(name="outs", bufs=2) as outpool,
        ):
            # ---------------- one-time constants ----------------
            ident = cpool.tile([128, 128], bf16)
            nc.sync.dma_start(ident[:], id_d[:])
            ones1 = cpool.tile([1, 128], f32)
            nc.vector.memset(ones1[:], 1.0)
            kb = cpool.tile([128, E], f32)
            for k in range(E):
                nc.vector.memset(kb[:, k:k + 1], float(k + 1))
            b_sb = cpool.tile([1, O_CORE], f32)
            nc.sync.dma_start(b_sb[:], b_d.rearrange("(p o) -> p o", p=1))
            br_sb = cpool.tile([1, E], f32)
            nc.sync.dma_start(br_sb[:], br_d.rearrange("(p o) -> p o", p=1))

            b_bcast = cpool.tile([128, O_CORE], f32)
            br_bcast = cpool.tile([128, E], f32)
            with tc.tile_pool(name="psum0", bufs=2, space="PSUM") as ps0:
                for j in range(O_CORE // 512):
                    t0 = ps0.tile([128, 512], f32)
                    nc.tensor.matmul(t0[:], ones1[:], b_sb[:, j * 512:(j + 1) * 512],
                                     start=True, stop=True)
                    nc.vector.tensor_copy(b_bcast[:, j * 512:(j + 1) * 512], t0[:])
                t1 = ps0.tile([128, E], f32, tag="ps0br")
                nc.tensor.matmul(t1[:], ones1[:], br_sb[:], start=True, stop=True)
                nc.vector.tensor_copy(br_bcast[:], t1[:])

            # comb[:, 136c : 136(c+1)] = [WrT_c (8) | A_cat_c (128)]
            comb = cpool.tile([128, NC_D * RH], bf16)
            comb3 = comb[:].rearrange("p (c f) -> p c f", f=RH)
            for e in range(E):
                nc.gpsimd.dma_start(
                    comb3[:, :, 8 + R * e: 8 + R * (e + 1)],
                    la_d[e].rearrange("(c p) r -> p c r", p=128),
                )
            # W_router: load [8,D] (pad to 16 rows), transpose per 128-col slice
            wr_nat = cpool.tile([16, D], bf16)
            nc.vector.memset(wr_nat[:], 0.0)
            nc.gpsimd.dma_start(wr_nat[0:8, :], wr_d[:])
            for c in range(NC_D):
                wrt_tmp = smpool.tile([128, 16], bf16, tag="wrt")
                nc.sync.dma_start(wrt_tmp[:], wr_nat[:, c * 128:(c + 1) * 128],
                                  transpose=True)
                nc.vector.tensor_copy(comb3[:, c, 0:8], wrt_tmp[:, 0:8])

            # lora_B cat: [er, o] bf16
            b_cat = cpool.tile([128, O_CORE], bf16)
            nc.gpsimd.dma_start(b_cat[:], lb_d[:])

            # ---------------- prefetch x tile 0 ----------------
            def load_xt(i):
                xstage = stpool.tile([128, D], bf16, tag="stage")
                nc.gpsimd.dma_start(xstage[:], x_d[i * 128:(i + 1) * 128, :])
                xt = xtpool.tile([128, NC_D * 128], bf16, tag="xt")
                for c in range(NC_D):
                    nc.sync.dma_start(xt[:, c * 128:(c + 1) * 128],
                                      xstage[:, c * 128:(c + 1) * 128],
                                      transpose=True)
                return xt

            xt0 = load_xt(0)

            # ---------------- resident W^T (bf16) ----------------
            # wt_all[:, c*O_CORE + o] = W[o, 128c + p]
            wt_all = cpool.tile([128, NC_D * O_CORE], bf16)
            for i in range(O_CORE // 128):
                wstage = stpool.tile([128, D], bf16, tag="stage")
                nc.gpsimd.dma_start(wstage[:], w_d[i * 128:(i + 1) * 128, :])
                for c in range(NC_D):
                    nc.sync.dma_start(
                        wt_all[:, c * O_CORE + i * 128: c * O_CORE + (i + 1) * 128],
                        wstage[:, c * 128:(c + 1) * 128],
                        transpose=True,
                    )

            # ---------------- main token loop ----------------
            with (
                tc.tile_pool(name="psrh", bufs=2, space="PSUM") as psrh,
                tc.tile_pool(name="pstr", bufs=2, space="PSUM") as pstr,
                tc.tile_pool(name="psog", bufs=2, space="PSUM") as psog,
            ):
                xt = xt0
                for i in range(NT):
                    # router + h fused matmul: [t,136]
                    rh = psrh.tile([128, RH], f32, tag="rh")
                    for c in range(NC_D):
                        nc.tensor.matmul(rh[:], xt[:, c * 128:(c + 1) * 128],
                                         comb[:, c * RH:(c + 1) * RH],
                                         start=(c == 0), stop=(c == NC_D - 1))

                    # sparsemax on logits
                    z = smpool.tile([128, E], f32, tag="z")
                    nc.vector.tensor_tensor(z[:], rh[:, 0:8], br_bcast[:], op=Add)
                    zs = smpool.tile([128, E], f32, tag="zs")
                    nc.vector.tensor_copy(zs[:], z[:])
                    tmp = smpool.tile([128, 1], f32, tag="tmp")
                    for (a_, b_) in SORT8:
                        ca, cb = zs[:, a_:a_ + 1], zs[:, b_:b_ + 1]
                        nc.vector.tensor_tensor(tmp[:], ca, cb, op=Min)
                        nc.vector.tensor_tensor(ca, ca, cb, op=Max)
                        nc.vector.tensor_copy(cb, tmp[:])
                    cum = smpool.tile([128, E], f32, tag="cum")
                    nc.vector.tensor_copy(cum[:, 0:1], zs[:, 0:1])
                    for k in range(1, E):
                        nc.vector.tensor_tensor(cum[:, k:k + 1], cum[:, k - 1:k],
                                                zs[:, k:k + 1], op=Add)
                    kz1 = smpool.tile([128, E], f32, tag="kz1")
                    nc.vector.tensor_tensor(kz1[:], zs[:], kb[:], op=Mult)
                    nc.vector.tensor_scalar_add(kz1[:], kz1[:], 1.0)
                    supp = smpool.tile([128, E], f32, tag="supp")
                    nc.vector.tensor_tensor(supp[:], kz1[:], cum[:], op=IsGt)
                    kz = smpool.tile([128, 1], f32, tag="kz")
                    nc.vector.tensor_reduce(kz[:], supp[:],
                                            axis=mybir.AxisListType.X, op=Add)
                    nc.vector.tensor_tensor(zs[:], zs[:], supp[:], op=Mult)
                    tsum = smpool.tile([128, 1], f32, tag="tsum")
                    nc.vector.tensor_reduce(tsum[:], zs[:],
                                            axis=mybir.AxisListType.X, op=Add)
                    nc.vector.tensor_scalar_add(tsum[:], tsum[:], -1.0)
                    rk = smpool.tile([128, 1], f32, tag="rk")
                    nc.vector.reciprocal(rk[:], kz[:])
                    tau = smpool.tile([128, 1], f32, tag="tau")
                    nc.vector.tensor_tensor(tau[:], tsum[:], rk[:], op=Mult)
                    wts = smpool.tile([128, E], f32, tag="wts")
                    nc.vector.tensor_scalar(wts[:], z[:], tau[:], None, op0=Sub)
                    nc.vector.tensor_scalar_max(wts[:], wts[:], 0.0)

                    # hw = h * w  (bf16), transpose via PE
                    hw = smpool.tile([128, ER], bf16, tag="hw")
                    for e in range(E):
                        nc.vector.tensor_scalar(
                            hw[:, e * R:(e + 1) * R], rh[:, 8 + e * R: 8 + (e + 1) * R],
                            wts[:, e:e + 1], None, op0=Mult)
                    tps = pstr.tile([128, 128], bf16, tag="tps")
                    nc.tensor.transpose(tps[:], hw[:], ident[:])
                    hwT = smpool.tile([128, ER], bf16, tag="hwT")
                    nc.vector.tensor_copy(hwT[:], tps[:])

                    # prefetch next x tile
                    xt_next = load_xt(i + 1) if i + 1 < NT else None

                    # base + lora matmuls, by o-group of 1024
                    for og in range(NOG):
                        acc = psog.tile([128, 1024], f32, tag="og")
                        for c in range(NC_D):
                            lhs = xt[:, c * 128:(c + 1) * 128]
                            base_col = c * O_CORE + og * 1024
                            nc.tensor.matmul(acc[:, 0:512], lhs,
                                             wt_all[:, base_col:base_col + 512],
                                             start=(c == 0), stop=False)
                            nc.tensor.matmul(acc[:, 512:1024], lhs,
                                             wt_all[:, base_col + 512:base_col + 1024],
                                             start=(c == 0), stop=False)
                        nc.tensor.matmul(acc[:, 0:512], hwT[:],
                                         b_cat[:, og * 1024: og * 1024 + 512],
                                         start=False, stop=True)
                        nc.tensor.matmul(acc[:, 512:1024], hwT[:],
                                         b_cat[:, og * 1024 + 512: (og + 1) * 1024],
                                         start=False, stop=True)
                        osb = outpool.tile([128, 1024], f32, tag="osb")
                        nc.vector.tensor_tensor(
                            osb[:], acc[:], b_bcast[:, og * 1024:(og + 1) * 1024],
                            op=Add)
                        nc.sync.dma_start(
                            out_d[i * 128:(i + 1) * 128, og * 1024:(og + 1) * 1024],
                            osb[:])
                    xt = xt_next

    nc.compile()
    _CACHE["nc"] = nc
    return nc


def make_in_maps(x, W_base, b_base, W_router, b_router, lora_A, lora_B):
    xf = np.ascontiguousarray(x.reshape(B * S, D), dtype=np.float32)
    ident = np.eye(128, dtype=ml_dtypes.bfloat16)
    lbf = lora_B.reshape(ER, O)
    in_maps = []
    for core in range(N_CORES):
        q, h = core % TQ, core // TQ
        in_maps.append({
            "x": xf[q * T_CORE:(q + 1) * T_CORE],
            "w": np.ascontiguousarray(W_base[h * O_CORE:(h + 1) * O_CORE]),
            "b": np.ascontiguousarray(b_base[h * O_CORE:(h + 1) * O_CORE]),
            "wr": np.ascontiguousarray(W_router),
            "br": np.ascontiguousarray(b_router),
            "la": np.ascontiguousarray(lora_A),
            "lb": np.ascontiguousarray(lbf[:, h * O_CORE:(h + 1) * O_CORE]),
            "ident": ident,
        })
    return in_maps


def assemble(results):
    out = np.empty((B * S, O), dtype=np.float32)
    for core in range(N_CORES):
        q, h = core % TQ, core // TQ
        out[q * T_CORE:(q + 1) * T_CORE,
            h * O_CORE:(h + 1) * O_CORE] = results[core]["out"]
    return out.reshape(B, S, O)


def kernel(x, W_base, b_base, W_router, b_router, lora_A, lora_B):
    nc = _build()
    in_maps = make_in_maps(x, W_base, b_base, W_router, b_router,
                           lora_A, lora_B)
    res = run_bass_kernel_spmd(nc, in_maps, core_ids=list(range(N_CORES)))
    return assemble(res.results)


if __name__ == "__main__":
    _build()
    print("kernel build+compile OK")


# revision 23
# speedup vs baseline: 56.2872x; 56.2872x over previous
"""Trainium2 Bass kernel for DGMoLE (dense-gated mixture of LoRA experts).

Computes, for x:[B,S,Din], W_base:[Dout,Din], b_base:[Dout], W_router:[E,Din],
b_router:[E], lora_A:[E,Din,R], lora_B:[E,R,Dout]:

    base   = x @ W_base.T + b_base
    wts    = sparsemax(x @ W_router.T + b_router)
    h      = einsum('td,edr->ter', x, lora_A)
    out    = base + einsum('ter,te,ero->to', h, wts, lora_B)

Sharding over 8 NeuronCores: 4 token-quarters x 2 Dout-halves.  Each core
holds its W_base half-transposed in SBUF as bf16 and streams its token
quarter through it.  All matmuls run in bf16 with fp32 PSUM accumulation.
"""

import sys

sys.path.insert(0, "/opt/trn_rl_repo")

import numpy as np
import ml_dtypes

from concourse import bacc, tile, mybir
from concourse.bass_utils import run_bass_kernel_spmd

f32 = mybir.dt.float32
bf16 = mybir.dt.bfloat16
Add = mybir.AluOpType.add
Mult = mybir.AluOpType.mult
Max = mybir.AluOpType.max
Min = mybir.AluOpType.min
IsGt = mybir.AluOpType.is_gt
Sub = mybir.AluOpType.subtract

# Problem dims (hardcoded per spec).
B, S, D, O = 8, 2048, 4096, 4096
E, R = 8, 16
ER = E * R  # 128
N_CORES = 8
TQ = 4          # token quarters
OH = 2          # output halves
T_CORE = B * S // TQ      # 4096 tokens per core
O_CORE = O // OH          # 2048 output dims per core
NT = T_CORE // 128        # 32 token tiles
NC_D = D // 128           # 32 contraction chunks
NOG = 2                   # o-groups of 1024 per core
RH = 8 + ER               # router+h fused rhs width = 136

# Batcher odd-even mergesort network for 8 elements (descending).
SORT8 = [(0, 1), (2, 3), (4, 5), (6, 7),
         (0, 2), (1, 3), (4, 6), (5, 7),
         (1, 2), (5, 6),
         (0, 4), (1, 5), (2, 6), (3, 7),
         (2, 4), (3, 5),
         (1, 2), (3, 4), (5, 6)]

_CACHE = {}


def _build(trace_sim=False):
    if "nc" in _CACHE:
        return _CACHE["nc"]

    nc = bacc.Bacc("TRN2", target_bir_lowering=False, debug=False,
                   num_devices=N_CORES)
    x_d = nc.dram_tensor("x", [T_CORE, D], f32, kind="ExternalInput").ap()
    w_d = nc.dram_tensor("w", [O_CORE, D], f32, kind="ExternalInput").ap()
    b_d = nc.dram_tensor("b", [O_CORE], f32, kind="ExternalInput").ap()
    wr_d = nc.dram_tensor("wr", [E, D], f32, kind="ExternalInput").ap()
    br_d = nc.dram_tensor("br", [E], f32, kind="ExternalInput").ap()
    la_d = nc.dram_tensor("la", [E, D, R], f32, kind="ExternalInput").ap()
    lb_d = nc.dram_tensor("lb", [ER, O_CORE], f32, kind="ExternalInput").ap()
    id_d = nc.dram_tensor("ident", [128, 128], bf16, kind="ExternalInput").ap()
    out_d = nc.dram_tensor("out", [T_CORE, O_CORE], f32,
                           kind="ExternalOutput").ap()

    with tile.TileContext(nc, trace_sim=trace_sim) as tc:
        with (
            tc.tile_pool(name="const", bufs=1) as cpool,
            tc.tile_pool(name="stage", bufs=2) as stpool,
            tc.tile_pool(name="xt", bufs=2) as xtpool,
            tc.tile_pool(name="small", bufs=2) as smpool,
            tc.tile_pool(name="outs", bufs=2) as outpool,
            tc.tile_pool(name="psrh", bufs=1, space="PSUM") as psrh,
            tc.tile_pool(name="pstr", bufs=3, space="PSUM") as pstr,
            tc.tile_pool(name="psog", bufs=2, space="PSUM") as psog,
        ):
            # ---------------- one-time constants ----------------
            ident = cpool.tile([128, 128], bf16)
            nc.sync.dma_start(ident[:], id_d[:])
            ones1 = cpool.tile([1, 128], f32)
            nc.vector.memset(ones1[:], 1.0)
            kb = cpool.tile([128, E], f32)
            for k in range(E):
                nc.vector.memset(kb[:, k:k + 1], float(k + 1))
            b_sb = cpool.tile([1, O_CORE], f32)
            nc.sync.dma_start(b_sb[:], b_d.rearrange("(p o) -> p o", p=1))
            br_sb = cpool.tile([1, E], f32)
            nc.sync.dma_start(br_sb[:], br_d.rearrange("(p o) -> p o", p=1))

            b_bcast = cpool.tile([128, O_CORE], f32)
            br_bcast = cpool.tile([128, E], f32)
            for j in range(O_CORE // 1024):
                t0 = psog.tile([128, 1024], f32, tag="og")
                for s2 in range(2):
                    nc.tensor.matmul(t0[:, s2 * 512:(s2 + 1) * 512], ones1[:],
                                     b_sb[:, j * 1024 + s2 * 512:
                                          j * 1024 + (s2 + 1) * 512],
                                     start=True, stop=True)
                nc.vector.tensor_copy(b_bcast[:, j * 1024:(j + 1) * 1024], t0[:])
            t1 = psrh.tile([128, RH], f32, tag="rh")
            nc.tensor.matmul(t1[:, 0:E], ones1[:], br_sb[:], start=True, stop=True)
            nc.vector.tensor_copy(br_bcast[:], t1[:, 0:E])

            # comb[:, 136c : 136(c+1)] = [WrT_c (8) | A_cat_c (128)]
            comb = cpool.tile([128, NC_D * RH], bf16)
            comb3 = comb[:].rearrange("p (c f) -> p c f", f=RH)
            for e in range(E):
                nc.gpsimd.dma_start(
                    comb3[:, :, 8 + R * e: 8 + R * (e + 1)],
                    la_d[e].rearrange("(c p) r -> p c r", p=128),
                )
            # W_router: load [8,D] (pad to 16 rows), transpose per 128-col slice
            wr_nat = stpool.tile([16, D], bf16, tag="stage")
            nc.vector.memset(wr_nat[:], 0.0)
            nc.gpsimd.dma_start(wr_nat[0:8, :], wr_d[:])
            wrt_all = smpool.tile([128, NC_D * 16], bf16, tag="wrt")
            wrt3 = wrt_all[:].rearrange("p (c r) -> p c r", r=16)
            nc.sync.dma_start_transpose(out=wrt3, in_=wr_nat[:])
            nc.vector.tensor_copy(comb3[:, :, 0:8], wrt3[:, :, 0:8])

            # lora_B cat: [er, o] bf16
            b_cat = cpool.tile([128, O_CORE], bf16)
            nc.gpsimd.dma_start(b_cat[:], lb_d[:])

            # ---------------- prefetch x tile 0 ----------------
            # All large transposes run on the PE (transpose-mode matmul with
            # identity rhs) + DVE evacuation: DMA-xbar transposes measured
            # ~77 GB/s with heavy serialization, PE does [128,128] bf16 in
            # ~60 ns.
            def pe_transpose(dst, src):
                tp = pstr.tile([128, 128], bf16, tag="tps")
                nc.tensor.transpose(tp[:], src, ident[:])
                nc.vector.tensor_copy(dst, tp[:])

            def load_stage(i):
                xstage = stpool.tile([128, D], bf16, tag="stage")
                nc.gpsimd.dma_start(xstage[:], x_d[i * 128:(i + 1) * 128, :])
                return xstage

            def transpose_stage(xstage):
                xt = xtpool.tile([128, NC_D * 128], bf16, tag="xt")
                for c in range(NC_D):
                    pe_transpose(xt[:, c * 128:(c + 1) * 128],
                                 xstage[:, c * 128:(c + 1) * 128])
                return xt

            xt0 = transpose_stage(load_stage(0))

            # ---------------- resident W^T (bf16) ----------------
            # wt_og[g][:, c*1024 + o] = W[g*1024 + o, 128c + p]; split per
            # o-group so og0 matmuls needn't wait for the full W build.
            wt_og = []
            for g in range(NOG):
                wt_g = cpool.tile([128, NC_D * 1024], bf16, tag=f"wt{g}")
                wt_og.append(wt_g)
            for i in range(O_CORE // 128):
                g, ii = i // (1024 // 128), i % (1024 // 128)
                wstage = stpool.tile([128, D], bf16, tag="stage")
                nc.gpsimd.dma_start(wstage[:], w_d[i * 128:(i + 1) * 128, :])
                for c in range(NC_D):
                    pe_transpose(
                        wt_og[g][:, c * 1024 + ii * 128: c * 1024 + (ii + 1) * 128],
                        wstage[:, c * 128:(c + 1) * 128])

            # ---------------- main token loop ----------------
            if True:
                xt = xt0
                for i in range(NT):
                    # router + h fused matmul: [t,136]
                    rh = psrh.tile([128, RH], f32, tag="rh")
                    for c in range(NC_D):
                        nc.tensor.matmul(rh[:], xt[:, c * 128:(c + 1) * 128],
                                         comb[:, c * RH:(c + 1) * RH],
                                         start=(c == 0), stop=(c == NC_D - 1))
                    # issue next tile's load now for DMA lead time; PE
                    # transposes for it are emitted after this tile's matmuls
                    stage_next = load_stage(i + 1) if i + 1 < NT else None

                    # sparsemax on logits
                    z = smpool.tile([128, E], f32, tag="z")
                    nc.vector.tensor_tensor(z[:], rh[:, 0:8], br_bcast[:], op=Add)
                    zs = smpool.tile([128, E], f32, tag="zs")
                    nc.vector.tensor_copy(zs[:], z[:])
                    tmp = smpool.tile([128, 1], f32, tag="tmp")
                    for (a_, b_) in SORT8:
                        ca, cb = zs[:, a_:a_ + 1], zs[:, b_:b_ + 1]
                        nc.vector.tensor_tensor(tmp[:], ca, cb, op=Min)
                        nc.vector.tensor_tensor(ca, ca, cb, op=Max)
                        nc.vector.tensor_copy(cb, tmp[:])
                    cum = smpool.tile([128, E], f32, tag="cum")
                    nc.vector.tensor_copy(cum[:, 0:1], zs[:, 0:1])
                    for k in range(1, E):
                        nc.vector.tensor_tensor(cum[:, k:k + 1], cum[:, k - 1:k],
                                                zs[:, k:k + 1], op=Add)
                    kz1 = smpool.tile([128, E], f32, tag="kz1")
                    nc.vector.tensor_tensor(kz1[:], zs[:], kb[:], op=Mult)
                    nc.vector.tensor_scalar_add(kz1[:], kz1[:], 1.0)
                    supp = smpool.tile([128, E], f32, tag="supp")
                    nc.vector.tensor_tensor(supp[:], kz1[:], cum[:], op=IsGt)
                    kz = smpool.tile([128, 1], f32, tag="kz")
                    nc.vector.tensor_reduce(kz[:], supp[:],
                                            axis=mybir.AxisListType.X, op=Add)
                    nc.vector.tensor_tensor(zs[:], zs[:], supp[:], op=Mult)
                    tsum = smpool.tile([128, 1], f32, tag="tsum")
                    nc.vector.tensor_reduce(tsum[:], zs[:],
                                            axis=mybir.AxisListType.X, op=Add)
                    nc.vector.tensor_scalar_add(tsum[:], tsum[:], -1.0)
                    rk = smpool.tile([128, 1], f32, tag="rk")
                    nc.vector.reciprocal(rk[:], kz[:])
                    tau = smpool.tile([128, 1], f32, tag="tau")
                    nc.vector.tensor_tensor(tau[:], tsum[:], rk[:], op=Mult)
                    wts = smpool.tile([128, E], f32, tag="wts")
                    nc.vector.tensor_scalar(wts[:], z[:], tau[:], None, op0=Sub)
                    nc.vector.tensor_scalar_max(wts[:], wts[:], 0.0)

                    # hw = h * w  (bf16), transpose via PE
                    hw = smpool.tile([128, ER], bf16, tag="hw")
                    for e in range(E):
                        nc.vector.tensor_scalar(
                            hw[:, e * R:(e + 1) * R], rh[:, 8 + e * R: 8 + (e + 1) * R],
                            wts[:, e:e + 1], None, op0=Mult)
                    hwT = smpool.tile([128, ER], bf16, tag="hwT")
                    pe_transpose(hwT[:], hw[:])

                    # base + lora matmuls, by o-group of 1024
                    for og in range(NOG):
                        acc = psog.tile([128, 1024], f32, tag="og")
                        for c in range(NC_D):
                            lhs = xt[:, c * 128:(c + 1) * 128]
                            base_col = c * 1024
                            nc.tensor.matmul(acc[:, 0:512], lhs,
                                             wt_og[og][:, base_col:base_col + 512],
                                             start=(c == 0), stop=False)
                            nc.tensor.matmul(acc[:, 512:1024], lhs,
                                             wt_og[og][:, base_col + 512:base_col + 1024],
                                             start=(c == 0), stop=False)
                        nc.tensor.matmul(acc[:, 0:512], hwT[:],
                                         b_cat[:, og * 1024: og * 1024 + 512],
                                         start=False, stop=True)
                        nc.tensor.matmul(acc[:, 512:1024], hwT[:],
                                         b_cat[:, og * 1024 + 512: (og + 1) * 1024],
                                         start=False, stop=True)
                        osb = outpool.tile([128, 1024], f32, tag="osb")
                        nc.vector.tensor_tensor(
                            osb[:], acc[:], b_bcast[:, og * 1024:(og + 1) * 1024],
                            op=Add)
                        nc.sync.dma_start(
                            out_d[i * 128:(i + 1) * 128, og * 1024:(og + 1) * 1024],
                            osb[:])
                    xt = (transpose_stage(stage_next)
                          if stage_next is not None else None)

    nc.compile()
    _CACHE["nc"] = nc
    return nc


def make_in_maps(x, W_base, b_base, W_router, b_router, lora_A, lora_B):
    xf = np.ascontiguousarray(x.reshape(B * S, D), dtype=np.float32)
    ident = np.eye(128, dtype=ml_dtypes.bfloat16)
    lbf = lora_B.reshape(ER, O)
    in_maps = []
    for core in range(N_CORES):
        q, h = core % TQ, core // TQ
        in_maps.append({
            "x": xf[q * T_CORE:(q + 1) * T_CORE],
            "w": np.ascontiguousarray(W_base[h * O_CORE:(h + 1) * O_CORE]),
            "b": np.ascontiguousarray(b_base[h * O_CORE:(h + 1) * O_CORE]),
            "wr": np.ascontiguousarray(W_router),
            "br": np.ascontiguousarray(b_router),
            "la": np.ascontiguousarray(lora_A),
            "lb": np.ascontiguousarray(lbf[:, h * O_CORE:(h + 1) * O_CORE]),
            "ident": ident,
        })
    return in_maps


def assemble(results):
    out = np.empty((B * S, O), dtype=np.float32)
    for core in range(N_CORES):
        q, h = core % TQ, core // TQ
        out[q * T_CORE:(q + 1) * T_CORE,
            h * O_CORE:(h + 1) * O_CORE] = results[core]["out"]
    return out.reshape(B, S, O)


def kernel(x, W_base, b_base, W_router, b_router, lora_A, lora_B):
    nc = _build()
    in_maps = make_in_maps(x, W_base, b_base, W_router, b_router,
                           lora_A, lora_B)
    res = run_bass_kernel_spmd(nc, in_maps, core_ids=list(range(N_CORES)))
    return assemble(res.results)


if __name__ == "__main__":
    _build()
    print("kernel build+compile OK")


# revision 24
# speedup vs baseline: 64.3861x; 1.1439x over previous
"""Trainium2 Bass kernel for DGMoLE (dense-gated mixture of LoRA experts).

Computes, for x:[B,S,Din], W_base:[Dout,Din], b_base:[Dout], W_router:[E,Din],
b_router:[E], lora_A:[E,Din,R], lora_B:[E,R,Dout]:

    base   = x @ W_base.T + b_base
    wts    = sparsemax(x @ W_router.T + b_router)
    h      = einsum('td,edr->ter', x, lora_A)
    out    = base + einsum('ter,te,ero->to', h, wts, lora_B)

Sharding over 8 NeuronCores: 4 token-quarters x 2 Dout-halves.  Each core
holds its W_base half-transposed in SBUF as bf16 and streams its token
quarter through it.  All matmuls run in bf16 with fp32 PSUM accumulation.
"""

import sys

sys.path.insert(0, "/opt/trn_rl_repo")

import numpy as np
import ml_dtypes

from concourse import bacc, tile, mybir
from concourse.bass_utils import run_bass_kernel_spmd

f32 = mybir.dt.float32
bf16 = mybir.dt.bfloat16
Add = mybir.AluOpType.add
Mult = mybir.AluOpType.mult
Max = mybir.AluOpType.max
Min = mybir.AluOpType.min
IsGt = mybir.AluOpType.is_gt
Sub = mybir.AluOpType.subtract

# Problem dims (hardcoded per spec).
B, S, D, O = 8, 2048, 4096, 4096
E, R = 8, 16
ER = E * R  # 128
N_CORES = 8
TQ = 4          # token quarters
OH = 2          # output halves
T_CORE = B * S // TQ      # 4096 tokens per core
O_CORE = O // OH          # 2048 output dims per core
NT = T_CORE // 128        # 32 token tiles
NC_D = D // 128           # 32 contraction chunks
NOG = 2                   # o-groups of 1024 per core
RH = 8 + ER               # router+h fused rhs width = 136

# Batcher odd-even mergesort network for 8 elements (descending).
SORT8 = [(0, 1), (2, 3), (4, 5), (6, 7),
         (0, 2), (1, 3), (4, 6), (5, 7),
         (1, 2), (5, 6),
         (0, 4), (1, 5), (2, 6), (3, 7),
         (2, 4), (3, 5),
         (1, 2), (3, 4), (5, 6)]

_CACHE = {}


def _build(trace_sim=False):
    if "nc" in _CACHE:
        return _CACHE["nc"]

    nc = bacc.Bacc("TRN2", target_bir_lowering=False, debug=False,
                   num_devices=N_CORES)
    x_d = nc.dram_tensor("x", [T_CORE, D], f32, kind="ExternalInput").ap()
    w_d = nc.dram_tensor("w", [O_CORE, D], f32, kind="ExternalInput").ap()
    b_d = nc.dram_tensor("b", [O_CORE], f32, kind="ExternalInput").ap()
    wr_d = nc.dram_tensor("wr", [E, D], f32, kind="ExternalInput").ap()
    br_d = nc.dram_tensor("br", [E], f32, kind="ExternalInput").ap()
    la_d = nc.dram_tensor("la", [E, D, R], f32, kind="ExternalInput").ap()
    lb_d = nc.dram_tensor("lb", [ER, O_CORE], f32, kind="ExternalInput").ap()
    id_d = nc.dram_tensor("ident", [128, 128], bf16, kind="ExternalInput").ap()
    out_d = nc.dram_tensor("out", [T_CORE, O_CORE], f32,
                           kind="ExternalOutput").ap()

    with tile.TileContext(nc, trace_sim=trace_sim) as tc:
        with (
            tc.tile_pool(name="const", bufs=1) as cpool,
            tc.tile_pool(name="stage", bufs=2) as stpool,
            tc.tile_pool(name="xt", bufs=2) as xtpool,
            tc.tile_pool(name="small", bufs=2) as smpool,
            tc.tile_pool(name="outs", bufs=2) as outpool,
            tc.tile_pool(name="psrh", bufs=2, space="PSUM") as psrh,
            tc.tile_pool(name="pstr", bufs=2, space="PSUM") as pstr,
            tc.tile_pool(name="psog", bufs=2, space="PSUM") as psog,
        ):
            # ---------------- one-time constants ----------------
            ident = cpool.tile([128, 128], bf16)
            nc.sync.dma_start(ident[:], id_d[:])
            ones1 = cpool.tile([1, 128], f32)
            nc.vector.memset(ones1[:], 1.0)
            kb = cpool.tile([128, E], f32)
            for k in range(E):
                nc.vector.memset(kb[:, k:k + 1], float(k + 1))
            b_sb = cpool.tile([1, O_CORE], f32)
            nc.sync.dma_start(b_sb[:], b_d.rearrange("(p o) -> p o", p=1))
            br_sb = cpool.tile([1, E], f32)
            nc.sync.dma_start(br_sb[:], br_d.rearrange("(p o) -> p o", p=1))

            b_bcast = cpool.tile([128, O_CORE], f32)
            br_bcast = cpool.tile([128, E], f32)
            for j in range(O_CORE // 1024):
                t0 = psog.tile([128, 1024], f32, tag="og")
                for s2 in range(2):
                    nc.tensor.matmul(t0[:, s2 * 512:(s2 + 1) * 512], ones1[:],
                                     b_sb[:, j * 1024 + s2 * 512:
                                          j * 1024 + (s2 + 1) * 512],
                                     start=True, stop=True)
                nc.vector.tensor_copy(b_bcast[:, j * 1024:(j + 1) * 1024], t0[:])
            t1 = psrh.tile([128, RH], f32, tag="rh")
            nc.tensor.matmul(t1[:, 0:E], ones1[:], br_sb[:], start=True, stop=True)
            nc.vector.tensor_copy(br_bcast[:], t1[:, 0:E])

            # comb[:, 136c : 136(c+1)] = [WrT_c (8) | A_cat_c (128)]
            comb = cpool.tile([128, NC_D * RH], bf16)
            comb3 = comb[:].rearrange("p (c f) -> p c f", f=RH)
            for e in range(E):
                nc.gpsimd.dma_start(
                    comb3[:, :, 8 + R * e: 8 + R * (e + 1)],
                    la_d[e].rearrange("(c p) r -> p c r", p=128),
                )
            # W_router: load [8,D] (pad to 16 rows), transpose per 128-col slice
            wr_nat = stpool.tile([16, D], bf16, tag="stage")
            nc.vector.memset(wr_nat[:], 0.0)
            nc.gpsimd.dma_start(wr_nat[0:8, :], wr_d[:])
            wrt_all = smpool.tile([128, NC_D * 16], bf16, tag="wrt")
            wrt3 = wrt_all[:].rearrange("p (c r) -> p c r", r=16)
            nc.sync.dma_start_transpose(out=wrt3, in_=wr_nat[:])
            nc.vector.tensor_copy(comb3[:, :, 0:8], wrt3[:, :, 0:8])

            # lora_B cat: [er, o] bf16
            b_cat = cpool.tile([128, O_CORE], bf16)
            nc.gpsimd.dma_start(b_cat[:], lb_d[:])

            # ---------------- prefetch x tile 0 ----------------
            # All large transposes run on the PE (transpose-mode matmul with
            # identity rhs) + DVE evacuation: DMA-xbar transposes measured
            # ~77 GB/s with heavy serialization, PE does [128,128] bf16 in
            # ~60 ns.
            def pe_transpose(dst, src):
                tp = pstr.tile([128, 128], bf16, tag="tps")
                nc.tensor.transpose(tp[:], src, ident[:])
                nc.vector.tensor_copy(dst, tp[:])

            def load_stage(i):
                xstage = stpool.tile([128, D], bf16, tag="stage")
                nc.gpsimd.dma_start(xstage[:], x_d[i * 128:(i + 1) * 128, :])
                return xstage

            def transpose_stage(xstage):
                xt = xtpool.tile([128, NC_D * 128], bf16, tag="xt")
                for c in range(NC_D):
                    pe_transpose(xt[:, c * 128:(c + 1) * 128],
                                 xstage[:, c * 128:(c + 1) * 128])
                return xt

            xt0 = transpose_stage(load_stage(0))

            # ---------------- resident W^T (bf16) ----------------
            # wt_og[g][:, c*1024 + o] = W[g*1024 + o, 128c + p]; split per
            # o-group so og0 matmuls needn't wait for the full W build.
            wt_og = []
            for g in range(NOG):
                wt_g = cpool.tile([128, NC_D * 1024], bf16, tag=f"wt{g}")
                wt_og.append(wt_g)
            for i in range(O_CORE // 128):
                g, ii = i // (1024 // 128), i % (1024 // 128)
                wstage = stpool.tile([128, D], bf16, tag="stage")
                nc.gpsimd.dma_start(wstage[:], w_d[i * 128:(i + 1) * 128, :])
                for c in range(NC_D):
                    pe_transpose(
                        wt_og[g][:, c * 1024 + ii * 128: c * 1024 + (ii + 1) * 128],
                        wstage[:, c * 128:(c + 1) * 128])

            # ---------------- main token loop ----------------
            if True:
                xt = xt0
                for i in range(NT):
                    # router + h fused matmul: [t,136]
                    rh = psrh.tile([128, RH], f32, tag="rh")
                    for c in range(NC_D):
                        nc.tensor.matmul(rh[:], xt[:, c * 128:(c + 1) * 128],
                                         comb[:, c * RH:(c + 1) * RH],
                                         start=(c == 0), stop=(c == NC_D - 1))
                    # issue next tile's load now for DMA lead time; PE
                    # transposes for it are emitted after this tile's matmuls
                    stage_next = load_stage(i + 1) if i + 1 < NT else None

                    # sparsemax on logits
                    z = smpool.tile([128, E], f32, tag="z")
                    nc.vector.tensor_tensor(z[:], rh[:, 0:8], br_bcast[:], op=Add)
                    zs = smpool.tile([128, E], f32, tag="zs")
                    nc.vector.tensor_copy(zs[:], z[:])
                    tmp = smpool.tile([128, 1], f32, tag="tmp")
                    for (a_, b_) in SORT8:
                        ca, cb = zs[:, a_:a_ + 1], zs[:, b_:b_ + 1]
                        nc.vector.tensor_tensor(tmp[:], ca, cb, op=Min)
                        nc.vector.tensor_tensor(ca, ca, cb, op=Max)
                        nc.vector.tensor_copy(cb, tmp[:])
                    cum = smpool.tile([128, E], f32, tag="cum")
                    nc.vector.tensor_copy(cum[:, 0:1], zs[:, 0:1])
                    for k in range(1, E):
                        nc.vector.tensor_tensor(cum[:, k:k + 1], cum[:, k - 1:k],
                                                zs[:, k:k + 1], op=Add)
                    kz1 = smpool.tile([128, E], f32, tag="kz1")
                    nc.vector.tensor_tensor(kz1[:], zs[:], kb[:], op=Mult)
                    nc.vector.tensor_scalar_add(kz1[:], kz1[:], 1.0)
                    supp = smpool.tile([128, E], f32, tag="supp")
                    nc.vector.tensor_tensor(supp[:], kz1[:], cum[:], op=IsGt)
                    kz = smpool.tile([128, 1], f32, tag="kz")
                    nc.vector.tensor_reduce(kz[:], supp[:],
                                            axis=mybir.AxisListType.X, op=Add)
                    nc.vector.tensor_tensor(zs[:], zs[:], supp[:], op=Mult)
                    tsum = smpool.tile([128, 1], f32, tag="tsum")
                    nc.vector.tensor_reduce(tsum[:], zs[:],
                                            axis=mybir.AxisListType.X, op=Add)
                    nc.vector.tensor_scalar_add(tsum[:], tsum[:], -1.0)
                    rk = smpool.tile([128, 1], f32, tag="rk")
                    nc.vector.reciprocal(rk[:], kz[:])
                    tau = smpool.tile([128, 1], f32, tag="tau")
                    nc.vector.tensor_tensor(tau[:], tsum[:], rk[:], op=Mult)
                    wts = smpool.tile([128, E], f32, tag="wts")
                    nc.vector.tensor_scalar(wts[:], z[:], tau[:], None, op0=Sub)
                    nc.vector.tensor_scalar_max(wts[:], wts[:], 0.0)

                    # hw = h * w  (bf16), transpose via PE
                    hw = smpool.tile([128, ER], bf16, tag="hw")
                    for e in range(E):
                        nc.vector.tensor_scalar(
                            hw[:, e * R:(e + 1) * R], rh[:, 8 + e * R: 8 + (e + 1) * R],
                            wts[:, e:e + 1], None, op0=Mult)
                    hwT = smpool.tile([128, ER], bf16, tag="hwT")
                    pe_transpose(hwT[:], hw[:])

                    # base + lora matmuls, by o-group of 1024
                    for og in range(NOG):
                        acc = psog.tile([128, 1024], f32, tag="og")
                        for c in range(NC_D):
                            lhs = xt[:, c * 128:(c + 1) * 128]
                            base_col = c * 1024
                            nc.tensor.matmul(acc[:, 0:512], lhs,
                                             wt_og[og][:, base_col:base_col + 512],
                                             start=(c == 0), stop=False)
                            nc.tensor.matmul(acc[:, 512:1024], lhs,
                                             wt_og[og][:, base_col + 512:base_col + 1024],
                                             start=(c == 0), stop=False)
                        nc.tensor.matmul(acc[:, 0:512], hwT[:],
                                         b_cat[:, og * 1024: og * 1024 + 512],
                                         start=False, stop=True)
                        nc.tensor.matmul(acc[:, 512:1024], hwT[:],
                                         b_cat[:, og * 1024 + 512: (og + 1) * 1024],
                                         start=False, stop=True)
                        osb = outpool.tile([128, 1024], f32, tag="osb")
                        nc.vector.tensor_tensor(
                            osb[:], acc[:], b_bcast[:, og * 1024:(og + 1) * 1024],
                            op=Add)
                        nc.sync.dma_start(
                            out_d[i * 128:(i + 1) * 128, og * 1024:(og + 1) * 1024],
                            osb[:])
                    xt = (transpose_stage(stage_next)
                          if stage_next is not None else None)

    nc.compile()
    _CACHE["nc"] = nc
    return nc


def make_in_maps(x, W_base, b_base, W_router, b_router, lora_A, lora_B):
    xf = np.ascontiguousarray(x.reshape(B * S, D), dtype=np.float32)
    ident = np.eye(128, dtype=ml_dtypes.bfloat16)
    lbf = lora_B.reshape(ER, O)
    in_maps = []
    for core in range(N_CORES):
        q, h = core % TQ, core // TQ
        in_maps.append({
            "x": xf[q * T_CORE:(q + 1) * T_CORE],
            "w": np.ascontiguousarray(W_base[h * O_CORE:(h + 1) * O_CORE]),
            "b": np.ascontiguousarray(b_base[h * O_CORE:(h + 1) * O_CORE]),
            "wr": np.ascontiguousarray(W_router),
            "br": np.ascontiguousarray(b_router),
            "la": np.ascontiguousarray(lora_A),
            "lb": np.ascontiguousarray(lbf[:, h * O_CORE:(h + 1) * O_CORE]),
            "ident": ident,
        })
    return in_maps


def assemble(results):
    out = np.empty((B * S, O), dtype=np.float32)
    for core in range(N_CORES):
        q, h = core % TQ, core // TQ
        out[q * T_CORE:(q + 1) * T_CORE,
            h * O_CORE:(h + 1) * O_CORE] = results[core]["out"]
    return out.reshape(B, S, O)


def kernel(x, W_base, b_base, W_router, b_router, lora_A, lora_B):
    nc = _build()
    in_maps = make_in_maps(x, W_base, b_base, W_router, b_router,
                           lora_A, lora_B)
    res = run_bass_kernel_spmd(nc, in_maps, core_ids=list(range(N_CORES)))
    return assemble(res.results)


if __name__ == "__main__":
    _build()
    print("kernel build+compile OK")
